# revision 4
# baseline (speedup 1.0000x reference)
"""AffineCouplingTransform forward on 8 TRN2 NeuronCores.

Reference computation (per row of x [B, 512]):
    id = x[:, :256]; tr = x[:, 256:]
    h = relu(id @ W1 + b1); h = relu(h @ W2 + b2); h = relu(h @ W3 + b3)
    params = h @ W4 + b4                    # [B, 512]
    shift = params[:, :256]; u = params[:, 256:]
    scale = sigmoid(u + 2) + 0.001
    out = concat([id, tr * scale + shift]); logabsdet = sum(log(scale), -1)

Sharding: pure data parallel over the batch dim (8192 rows per core),
weights replicated. Matmuls run in float32r (TF32-like fp32 PE mode,
1 cycle/row at N=512 — 4x faster than plain fp32, ~16x more accurate
than bf16; measured max rel err vs scale ~2e-4 on HW). Activations are
kept feature-major ([features-on-partitions, batch-free]) through the
MLP so no transposes are needed between layers; the input x_id block is
transposed on the PE, and the last layer swaps matmul operand roles
(activations stationary, W4 moving) so params come out batch-major and
the epilogue + output DMA need no transposes at all.
"""

import numpy as np

import concourse.bacc as bacc
import concourse.bass as bass
import concourse.mybir as mybir
import concourse.tile as tile
from concourse.bass_utils import run_bass_kernel_spmd
from concourse.masks import make_identity

B = 65536
D = 512
D_ID = 256
D_TR = 256
H = 1024
N_CORES = 8
B_CORE = B // N_CORES  # 8192
NT = 512  # batch columns processed per superchunk (max fp32 moving free dim)
NSC = B_CORE // NT  # 16 superchunks per core
NSUB = NT // 128  # 4 batch subchunks of 128 rows

F32 = mybir.dt.float32
F32R = mybir.dt.float32r
RELU = mybir.ActivationFunctionType.Relu
SIGMOID = mybir.ActivationFunctionType.Sigmoid
LN = mybir.ActivationFunctionType.Ln
ADD = mybir.AluOpType.add
MULT = mybir.AluOpType.mult


def build_nc():
    nc = bacc.Bacc(None)

    x = nc.dram_tensor("x", [B_CORE, D], F32, kind="ExternalInput")
    w1 = nc.dram_tensor("w1", [D_ID, H], F32R, kind="ExternalInput")
    b1 = nc.dram_tensor("b1", [H], F32, kind="ExternalInput")
    w2 = nc.dram_tensor("w2", [H, H], F32R, kind="ExternalInput")
    b2 = nc.dram_tensor("b2", [H], F32, kind="ExternalInput")
    w3 = nc.dram_tensor("w3", [H, H], F32R, kind="ExternalInput")
    b3 = nc.dram_tensor("b3", [H], F32, kind="ExternalInput")
    w4 = nc.dram_tensor("w4", [H, 2 * D_TR], F32R, kind="ExternalInput")
    b4 = nc.dram_tensor("b4", [2 * D_TR], F32, kind="ExternalInput")
    out = nc.dram_tensor("out", [B_CORE, D], F32, kind="ExternalOutput")
    logdet = nc.dram_tensor("logdet", [B_CORE], F32, kind="ExternalOutput")

    with tile.TileContext(nc) as tc:
        with (
            tc.tile_pool(name="wpool", bufs=1) as wpool,
            tc.tile_pool(name="xio", bufs=2) as xio,
            tc.tile_pool(name="hbuf", bufs=1) as hbuf,
            tc.tile_pool(name="small", bufs=3) as small,
            tc.tile_pool(name="psl", bufs=4, space="PSUM") as psl,
            tc.tile_pool(name="pst", bufs=2, space="PSUM") as pst,
        ):
            # ---- resident weights / biases -------------------------------
            w1_sb = wpool.tile([128, D_ID // 128, H], F32R)
            w2_sb = wpool.tile([128, H // 128, H], F32R)
            w3_sb = wpool.tile([128, H // 128, H], F32R)
            w4_sb = wpool.tile([128, H // 128, 2 * D_TR], F32R)
            nc.sync.dma_start(w1_sb[:], w1.rearrange("(ko p) m -> p ko m", p=128))
            nc.sync.dma_start(w2_sb[:], w2.rearrange("(ko p) m -> p ko m", p=128))
            nc.sync.dma_start(w3_sb[:], w3.rearrange("(ko p) m -> p ko m", p=128))
            nc.sync.dma_start(w4_sb[:], w4.rearrange("(ko p) m -> p ko m", p=128))

            b1_sb = wpool.tile([128, H // 128], F32)
            b2_sb = wpool.tile([128, H // 128], F32)
            b3_sb = wpool.tile([128, H // 128], F32)
            nc.sync.dma_start(b1_sb[:], b1.rearrange("(mo p) -> p mo", p=128))
            nc.sync.dma_start(b2_sb[:], b2.rearrange("(mo p) -> p mo", p=128))
            nc.sync.dma_start(b3_sb[:], b3.rearrange("(mo p) -> p mo", p=128))

            b4_row = wpool.tile([1, 2 * D_TR], F32)
            nc.sync.dma_start(b4_row[:], b4[None, :])
            b4_all = wpool.tile([128, 2 * D_TR], F32)
            nc.gpsimd.partition_broadcast(b4_all[:], b4_row[:])

            ident = wpool.tile([128, 128], F32)
            make_identity(nc, ident[:])

            c_two = wpool.tile([128, 1], F32)
            nc.gpsimd.memset(c_two[:], 2.0)
            c_eps = wpool.tile([128, 1], F32)
            nc.gpsimd.memset(c_eps[:], 0.001)

            # identity half of the output never touches compute
            nc.sync.dma_start(out[:, :D_ID], x[:, :D_ID])

            # ---- main loop over superchunks of NT batch rows -------------
            for sc in range(NSC):
                rows = slice(sc * NT, (sc + 1) * NT)
                x_sb = xio.tile([128, NSUB, D], F32, tag="x")
                nc.sync.dma_start(
                    x_sb[:], x[rows, :].rearrange("(i p) d -> p i d", p=128)
                )

                # transpose x_id into feature-major a0 [256-feat, NT-batch]
                a0 = xio.tile([128, D_ID // 128, NT], F32R, tag="a0")
                for i in range(NSUB):
                    for ks in range(D_ID // 128):
                        pt = pst.tile([128, 128], F32, tag="tr")
                        nc.tensor.transpose(
                            pt[:], x_sb[:, i, ks * 128 : (ks + 1) * 128], ident[:]
                        )
                        nc.vector.tensor_copy(
                            a0[:, ks, i * 128 : (i + 1) * 128], pt[:]
                        )

                # three hidden layers, activations stay feature-major f32r
                h_prev = a0
                for w_sb, b_sb, htag in (
                    (w1_sb, b1_sb, "h1"),
                    (w2_sb, b2_sb, "h2"),
                    (w3_sb, b3_sb, "h3"),
                ):
                    ko_n = h_prev.shape[1]
                    h_cur = hbuf.tile([128, H // 128, NT], F32R, tag=htag)
                    for mo in range(H // 128):
                        ps = psl.tile([128, NT], F32, tag="ps")
                        for ko in range(ko_n):
                            nc.tensor.matmul(
                                ps[:],
                                w_sb[:, ko, mo * 128 : (mo + 1) * 128],
                                h_prev[:, ko, :],
                                start=(ko == 0),
                                stop=(ko == ko_n - 1),
                            )
                        nc.scalar.activation(
                            h_cur[:, mo, :], ps[:], RELU, bias=b_sb[:, mo : mo + 1]
                        )
                    h_prev = h_cur

                # last layer with swapped operands: params batch-major
                ld_sb = small.tile([128, NSUB], F32, tag="ld")
                for i in range(NSUB):
                    ps4 = psl.tile([128, 2 * D_TR], F32, tag="ps")
                    for ko in range(H // 128):
                        nc.tensor.matmul(
                            ps4[:],
                            h_prev[:, ko, i * 128 : (i + 1) * 128],
                            w4_sb[:, ko, :],
                            start=(ko == 0),
                            stop=(ko == H // 128 - 1),
                        )
                    # epilogue, all batch-major [128 rows, 256]
                    u_sb = small.tile([128, D_TR], F32, tag="u")
                    nc.vector.tensor_add(u_sb[:], ps4[:, D_TR:], b4_all[:, D_TR:])
                    sig = small.tile([128, D_TR], F32, tag="sig")
                    nc.scalar.activation(sig[:], u_sb[:], SIGMOID, bias=c_two[:])
                    lntmp = small.tile([128, D_TR], F32, tag="lntmp")
                    nc.scalar.activation(
                        lntmp[:],
                        sig[:],
                        LN,
                        bias=c_eps[:],
                        accum_out=ld_sb[:, i : i + 1],
                    )
                    shift = small.tile([128, D_TR], F32, tag="shift")
                    nc.vector.tensor_add(shift[:], ps4[:, :D_TR], b4_all[:, :D_TR])
                    t_sb = small.tile([128, D_TR], F32, tag="t")
                    nc.vector.scalar_tensor_tensor(
                        t_sb[:],
                        sig[:],
                        0.001,
                        x_sb[:, i, D_ID:],
                        op0=ADD,
                        op1=MULT,
                    )
                    otr = small.tile([128, D_TR], F32, tag="otr")
                    nc.vector.tensor_add(otr[:], t_sb[:], shift[:])
                    nc.sync.dma_start(
                        out[sc * NT + i * 128 : sc * NT + (i + 1) * 128, D_ID:],
                        otr[:],
                    )
                nc.sync.dma_start(
                    logdet[rows].rearrange("(i p) -> p i", p=128), ld_sb[:]
                )

    nc.finalize()
    return nc


_NC_CACHE = []


def _get_nc():
    if not _NC_CACHE:
        _NC_CACHE.append(build_nc())
    return _NC_CACHE[0]


def kernel(x, W1, b1, W2, b2, W3, b3, W4, b4):
    f = np.ascontiguousarray
    x = f(np.asarray(x, dtype=np.float32))
    shared = {
        "w1": f(np.asarray(W1, np.float32)),
        "b1": f(np.asarray(b1, np.float32)),
        "w2": f(np.asarray(W2, np.float32)),
        "b2": f(np.asarray(b2, np.float32)),
        "w3": f(np.asarray(W3, np.float32)),
        "b3": f(np.asarray(b3, np.float32)),
        "w4": f(np.asarray(W4, np.float32)),
        "b4": f(np.asarray(b4, np.float32)),
    }
    in_maps = [
        {"x": x[c * B_CORE : (c + 1) * B_CORE], **shared} for c in range(N_CORES)
    ]
    nc = _get_nc()
    res = run_bass_kernel_spmd(nc, in_maps, core_ids=list(range(N_CORES)))
    outputs = np.concatenate([r["out"] for r in res.results], axis=0)
    logabsdet = np.concatenate([r["logdet"] for r in res.results], axis=0)
    return outputs, logabsdet


# revision 28
# speedup vs baseline: 15412.8489x; 15412.8489x over previous
"""AffineCouplingTransform forward on 8 TRN2 NeuronCores.

Reference computation (per row of x [B, 512]):
    id = x[:, :256]; tr = x[:, 256:]
    h = relu(id @ W1 + b1); h = relu(h @ W2 + b2); h = relu(h @ W3 + b3)
    params = h @ W4 + b4                    # [B, 512]
    shift = params[:, :256]; u = params[:, 256:]
    scale = sigmoid(u + 2) + 0.001
    out = concat([id, tr * scale + shift]); logabsdet = sum(log(scale), -1)

Sharding: pure data parallel over the batch dim (8192 rows per core),
weights replicated. Matmuls run in float16 (HW-measured 212 ns per
128x128x512 MM — same as bf16 since fast-weight-load works for 2-byte
dtypes, vs 327 ns for float32r whose 4-byte LDWEIGHTS cannot be hidden
— but with a 10-bit mantissa; all activations/weights here are O(1) so
fp16 range is safe, end-to-end rel err ~2e-5). Activations are kept
feature-major ([features-on-partitions, batch-free]) through the MLP so
no transposes are needed between layers; the x_id block is transposed
by XBAR DMA engines from a host-cast fp16 copy (zero PE/DVE cost, on
the ACT HWDGE queue so neither DMA queue ever switches xbar mode), and
the last layer swaps matmul operand roles (activations stationary, W4
moving) so params come out batch-major and the epilogue + output DMA
need no transposes at all.
"""

import numpy as np

import concourse.bacc as bacc
import concourse.bass as bass
import concourse.mybir as mybir
import concourse.tile as tile
from concourse.bass_utils import run_bass_kernel_spmd

B = 65536
D = 512
D_ID = 256
D_TR = 256
H = 1024
N_CORES = 8
B_CORE = B // N_CORES  # 8192
NT = 512  # batch columns processed per superchunk (max fp32 moving free dim)
NSC = B_CORE // NT  # 16 superchunks per core
NSUB = NT // 128  # 4 batch subchunks of 128 rows

F32 = mybir.dt.float32
F32R = mybir.dt.float32r
import os as _os
MM_DT = getattr(mybir.dt, _os.environ.get("KERNEL_MM_DT", "float16"))
RELU = mybir.ActivationFunctionType.Relu
SIGMOID = mybir.ActivationFunctionType.Sigmoid
LN = mybir.ActivationFunctionType.Ln
ADD = mybir.AluOpType.add
MULT = mybir.AluOpType.mult


def build_nc(timing_r=None):
    """timing_r: if set, build a timing variant — all big tensors Internal
    (no host transfer) and the whole compute wrapped in a tc.For_i hardware
    loop run timing_r times, so wall-clock deltas between two values of
    timing_r isolate pure on-device compute time."""
    nc = bacc.Bacc(None)

    kin = "Internal" if timing_r else "ExternalInput"
    kout = "Internal" if timing_r else "ExternalOutput"
    x = nc.dram_tensor("x", [B_CORE, D], F32, kind=kin)
    xid16 = nc.dram_tensor("xid16", [B_CORE, D_ID], MM_DT, kind=kin)
    w1 = nc.dram_tensor("w1", [D_ID, H], MM_DT, kind=kin)
    b1 = nc.dram_tensor("b1", [H], F32, kind=kin)
    w2 = nc.dram_tensor("w2", [H, H], MM_DT, kind=kin)
    b2 = nc.dram_tensor("b2", [H], F32, kind=kin)
    w3 = nc.dram_tensor("w3", [H, H], MM_DT, kind=kin)
    b3 = nc.dram_tensor("b3", [H], F32, kind=kin)
    w4 = nc.dram_tensor("w4", [H, 2 * D_TR], MM_DT, kind=kin)
    b4 = nc.dram_tensor("b4", [2 * D_TR], F32, kind=kin)
    out = nc.dram_tensor("out", [B_CORE, D], F32, kind=kout)
    logdet = nc.dram_tensor("logdet", [B_CORE], F32, kind="ExternalOutput")

    with tile.TileContext(nc) as tc:
        with (
            tc.tile_pool(name="wpool", bufs=1) as wpool,
            tc.tile_pool(name="xio", bufs=2) as xio,
            tc.tile_pool(name="hbuf", bufs=1) as hbuf,
            tc.tile_pool(name="small", bufs=3) as small,
            tc.tile_pool(name="psl", bufs=6, space="PSUM") as psl,
            tc.tile_pool(name="pst", bufs=2, space="PSUM") as pst,
        ):
            # ---- resident weights / biases -------------------------------
            # issue order matters: first x superchunk + W1 land first so PE
            # can start immediately; W2/W3/W4 stream in per-m-chunk behind.
            x_tiles = {}

            def load_x(sc):
                x_sb = xio.tile(
                    [128, NSUB, D], F32, tag="x", name=f"x_sb{sc}", bufs=3
                )
                nc.sync.dma_start(
                    x_sb[:],
                    x[sc * NT : (sc + 1) * NT, :].rearrange(
                        "(i p) d -> p i d", p=128
                    ),
                )
                x_tiles[sc] = x_sb

            a0_tiles = {}

            def load_a0(sc):
                # XBAR DMA-transpose of the host-cast fp16 x_id block into
                # feature-major [256-feat, NT-batch]; no PE/DVE involved
                a0 = xio.tile(
                    [128, D_ID // 128, NT], MM_DT, tag="a0", name=f"a0_{sc}",
                    bufs=3,
                )
                for ks in range(D_ID // 128):
                    nc.scalar.dma_start_transpose(
                        a0[:, ks, :],
                        xid16[sc * NT : (sc + 1) * NT, ks * 128 : (ks + 1) * 128],
                    )
                a0_tiles[sc] = a0

            load_a0(0)
            load_x(0)
            load_a0(1)

            w1_sb = wpool.tile([128, D_ID // 128, H], MM_DT)
            nc.sync.dma_start(w1_sb[:], w1.rearrange("(ko p) m -> p ko m", p=128))

            b1_sb = wpool.tile([128, H // 128], F32)
            b2_sb = wpool.tile([128, H // 128], F32)
            b3_sb = wpool.tile([128, H // 128], F32)
            nc.sync.dma_start(b1_sb[:], b1.rearrange("(mo p) -> p mo", p=128))
            nc.sync.dma_start(b2_sb[:], b2.rearrange("(mo p) -> p mo", p=128))
            nc.sync.dma_start(b3_sb[:], b3.rearrange("(mo p) -> p mo", p=128))

            b4_row = wpool.tile([1, 2 * D_TR], F32)
            nc.sync.dma_start(b4_row[:], b4[None, :])
            b4_all = wpool.tile([128, 2 * D_TR], F32)
            nc.gpsimd.partition_broadcast(b4_all[:], b4_row[:])

            c_two = wpool.tile([128, 1], F32)
            nc.gpsimd.memset(c_two[:], 2.0)
            c_eps = wpool.tile([128, 1], F32)
            nc.gpsimd.memset(c_eps[:], 0.001)

            # big weights after W1/x0, chunked by output column block so the
            # first consumers unblock progressively
            w2_sb = wpool.tile([128, H // 128, H], MM_DT)
            w3_sb = wpool.tile([128, H // 128, H], MM_DT)
            w4_sb = wpool.tile([128, H // 128, 2 * D_TR], MM_DT)
            w2_r = w2.rearrange("(ko p) m -> p ko m", p=128)
            w3_r = w3.rearrange("(ko p) m -> p ko m", p=128)
            for mo in range(H // 128):
                nc.sync.dma_start(
                    w2_sb[:, :, mo * 128 : (mo + 1) * 128],
                    w2_r[:, :, mo * 128 : (mo + 1) * 128],
                )
                if mo == 0:
                    load_x(1)
            for mo in range(H // 128):
                nc.sync.dma_start(
                    w3_sb[:, :, mo * 128 : (mo + 1) * 128],
                    w3_r[:, :, mo * 128 : (mo + 1) * 128],
                )
            nc.sync.dma_start(w4_sb[:], w4.rearrange("(ko p) m -> p ko m", p=128))

            # ---- main loop over superchunks of NT batch rows -------------
            from contextlib import ExitStack

            loop_ctx = ExitStack()
            if timing_r:
                loop_ctx.enter_context(tc.For_i(0, timing_r, 1))
            for sc in range(NSC):
                rows = slice(sc * NT, (sc + 1) * NT)
                x_sb = x_tiles[sc]
                if sc + 2 <= NSC - 1:
                    load_x(sc + 2)
                    load_a0(sc + 2)
                a0 = a0_tiles[sc]

                # three hidden layers, activations stay feature-major f32r
                h_prev = a0
                for w_sb, b_sb, htag in (
                    (w1_sb, b1_sb, "h1"),
                    (w2_sb, b2_sb, "h2"),
                    (w3_sb, b3_sb, "h3"),
                ):
                    ko_n = h_prev.shape[1]
                    h_cur = hbuf.tile([128, H // 128, NT], MM_DT, tag=htag)
                    for mo in range(H // 128):
                        ps = psl.tile([128, NT], F32, tag="ps")
                        for ko in range(ko_n):
                            nc.tensor.matmul(
                                ps[:],
                                w_sb[:, ko, mo * 128 : (mo + 1) * 128],
                                h_prev[:, ko, :],
                                start=(ko == 0),
                                stop=(ko == ko_n - 1),
                            )
                        if ko_n == 2 and mo < 6:
                            # L1's matmuls outrun a single drain engine;
                            # alternate the relu+bias between ACT and DVE
                            nc.vector.scalar_tensor_tensor(
                                h_cur[:, mo, :],
                                ps[:],
                                b_sb[:, mo : mo + 1],
                                nc.const_aps.tensor(0.0, (128, NT)),
                                op0=ADD,
                                op1=mybir.AluOpType.max,
                            )
                        else:
                            nc.scalar.activation(
                                h_cur[:, mo, :],
                                ps[:],
                                RELU,
                                bias=b_sb[:, mo : mo + 1],
                            )
                    h_prev = h_cur

                # identity half of the output never touches compute; SWDGE
                # queue keeps it off the SP queue that feeds x loads
                nc.gpsimd.dma_start(out[rows, :D_ID], x[rows, :D_ID])

                # last layer with swapped operands: params batch-major.
                # PSUM is drained (u/shift bias adds on DVE) right after each
                # subchunk's matmuls; sigmoids and lns are batched so the ACT
                # function table gets swapped 3x per superchunk instead of 8x.
                ld_sb = small.tile([128, NSUB], F32, tag="ld")
                for i in range(NSUB):
                    ps4 = psl.tile([128, 2 * D_TR], F32, tag="ps")
                    for ko in range(H // 128):
                        nc.tensor.matmul(
                            ps4[:],
                            h_prev[:, ko, i * 128 : (i + 1) * 128],
                            w4_sb[:, ko, :],
                            start=(ko == 0),
                            stop=(ko == H // 128 - 1),
                        )
                    u_sb = small.tile([128, D_TR], F32, tag="u")
                    nc.vector.tensor_add(u_sb[:], ps4[:, D_TR:], b4_all[:, D_TR:])
                    sig = small.tile([128, D_TR], F32, tag="sig")
                    nc.scalar.activation(sig[:], u_sb[:], SIGMOID, bias=c_two[:])
                    lntmp = small.tile([128, D_TR], F32, tag="lntmp")
                    nc.scalar.activation(
                        lntmp[:],
                        sig[:],
                        LN,
                        bias=c_eps[:],
                        accum_out=ld_sb[:, i : i + 1],
                    )
                    shift = small.tile([128, D_TR], F32, tag="shift")
                    nc.vector.tensor_add(shift[:], ps4[:, :D_TR], b4_all[:, :D_TR])
                    t_sb = small.tile([128, D_TR], F32, tag="t")
                    nc.vector.scalar_tensor_tensor(
                        t_sb[:],
                        sig[:],
                        0.001,
                        x_sb[:, i, D_ID:],
                        op0=ADD,
                        op1=MULT,
                    )
                    otr = small.tile([128, D_TR], F32, tag="otr")
                    nc.gpsimd.tensor_add(otr[:], t_sb[:], shift[:])
                    nc.sync.dma_start(
                        out[sc * NT + i * 128 : sc * NT + (i + 1) * 128, D_ID:],
                        otr[:],
                    )
                nc.sync.dma_start(
                    logdet[rows].rearrange("(i p) -> p i", p=128), ld_sb[:]
                )
            loop_ctx.close()

    nc.finalize()
    return nc


_NC_CACHE = []


def _get_nc():
    if not _NC_CACHE:
        _NC_CACHE.append(build_nc())
    return _NC_CACHE[0]


def kernel(x, W1, b1, W2, b2, W3, b3, W4, b4):
    f = np.ascontiguousarray
    x = f(np.asarray(x, dtype=np.float32))
    wdt = mybir.dt.np(MM_DT)
    shared = {
        "w1": f(np.asarray(W1, wdt)),
        "b1": f(np.asarray(b1, np.float32)),
        "w2": f(np.asarray(W2, wdt)),
        "b2": f(np.asarray(b2, np.float32)),
        "w3": f(np.asarray(W3, wdt)),
        "b3": f(np.asarray(b3, np.float32)),
        "w4": f(np.asarray(W4, wdt)),
        "b4": f(np.asarray(b4, np.float32)),
    }
    xid16 = np.ascontiguousarray(x[:, :D_ID].astype(np.float16))
    in_maps = [
        {
            "x": x[c * B_CORE : (c + 1) * B_CORE],
            "xid16": xid16[c * B_CORE : (c + 1) * B_CORE],
            **shared,
        }
        for c in range(N_CORES)
    ]
    nc = _get_nc()
    res = run_bass_kernel_spmd(nc, in_maps, core_ids=list(range(N_CORES)))
    outputs = np.concatenate([r["out"] for r in res.results], axis=0)
    logabsdet = np.concatenate([r["logdet"] for r in res.results], axis=0)
    return outputs, logabsdet
